# revision 1
# baseline (speedup 1.0000x reference)
"""GCN (2x GraphConv + BatchNorm) on 8 Trainium2 NeuronCores.

Sharding: 1D node partition (12500 dst-nodes per core). Edges are grouped by
dst shard on host (index preprocessing); each core gathers source features
from a replicated transformed-feature table, scatter-adds into its local node
block, and BN statistics are combined with psum collectives. Layer-2 input is
assembled with an all_gather.
"""
import numpy as np
from functools import partial

N = 100000
E = 1600000
F = 128
H = 64
EPS = 1e-5
NC = 8
NS = N // NC  # 12500 nodes per core


def _prep(src, dst):
    deg_out = np.bincount(src, minlength=N).astype(np.float32)
    deg_in = np.bincount(dst, minlength=N)
    norm_src = 1.0 / np.sqrt(np.maximum(deg_out, 1.0))
    norm_dst = 1.0 / np.sqrt(np.maximum(deg_in.astype(np.float32), 1.0))
    # Padded in-edge table: pad_idx[v, k] = src of k-th in-edge of v, N = dummy.
    # Aggregation then becomes K gathers + adds (no scatter, which the axon
    # backend cannot execute at scale).
    order = np.argsort(dst, kind="stable")
    s_sorted = src[order]
    d_sorted = dst[order]
    K = int(deg_in.max())
    offs = np.concatenate([[0], np.cumsum(deg_in)]).astype(np.int64)
    pos = np.arange(E, dtype=np.int64) - offs[d_sorted]
    pad_idx = np.full((N, K), N, np.int32)
    pad_idx[d_sorted, pos] = s_sorted
    return norm_src, norm_dst.reshape(NC, NS), pad_idx.reshape(NC, NS, K), K


_RUN_CACHE = {}


def _get_run(K):
    if K in _RUN_CACHE:
        return _RUN_CACHE[K]
    import jax
    import jax.numpy as jnp

    devs = jax.devices()[:NC]
    assert len(devs) == NC

    @partial(jax.pmap, axis_name="x", devices=devs)
    def run(features, norm_src, pad_idx, norm_dst_l,
            W1, b1, g1, be1, W2, b2_, g2, be2):
        def conv(x_full, W, b, ndl):
            h = jnp.dot(x_full * norm_src[:, None], W,
                        precision=jax.lax.Precision.HIGHEST)
            hz = jnp.concatenate([h, jnp.zeros((1, H), jnp.float32)], 0)
            agg = jnp.zeros((NS, H), jnp.float32)
            for k in range(K):
                agg = agg + hz[pad_idx[:, k]]
            return jax.nn.elu(agg * ndl[:, None] + b)

        def bn(xl, gamma, beta):
            mean = jax.lax.psum(xl.sum(0), "x") / N
            var = jax.lax.psum(jnp.square(xl - mean).sum(0), "x") / N
            return (xl - mean) * jax.lax.rsqrt(var + EPS) * gamma + beta

        h1 = bn(conv(features, W1, b1, norm_dst_l), g1, be1)
        h1_full = jax.lax.all_gather(h1, "x").reshape(N, H)
        h2 = bn(conv(h1_full, W2, b2_, norm_dst_l), g2, be2)
        return h2

    _RUN_CACHE[K] = run
    return run


def _device_impl(features, W1, b1, gamma1, beta1, W2, b2, gamma2, beta2,
                 norm_src, norm_dst_sh, pad_idx, K):
    run = _get_run(K)
    rep = lambda a: np.broadcast_to(a, (NC,) + a.shape)
    out = run(rep(features), rep(norm_src), pad_idx, norm_dst_sh,
              rep(W1), rep(b1), rep(gamma1), rep(beta1),
              rep(W2), rep(b2), rep(gamma2), rep(beta2))
    return np.asarray(out).reshape(N, H)


def _host_impl(features, W1, b1, gamma1, beta1, W2, b2, gamma2, beta2,
               src, dst, norm_src, norm_dst):
    def conv(x, W, b):
        h = (x * norm_src[:, None]) @ W
        order = np.argsort(dst, kind="stable")
        d_sorted = dst[order]
        msgs = h[src[order]]
        agg = np.zeros((N, h.shape[1]), np.float32)
        starts = np.searchsorted(d_sorted, np.arange(N))
        np.add.reduceat(msgs, starts, axis=0, out=agg)
        agg[np.diff(np.concatenate([starts, [E]])) == 0] = 0
        out = agg * norm_dst[:, None] + b
        return np.where(out > 0, out, np.expm1(np.minimum(out, 0)))

    def bn(x, gamma, beta):
        mean = x.mean(0)
        var = np.square(x - mean).mean(0)
        return (x - mean) / np.sqrt(var + EPS) * gamma + beta

    h1 = bn(conv(features, W1, b1), gamma1, beta1)
    return bn(conv(h1, W2, b2), gamma2, beta2)


def kernel(features, W1, b1, gamma1, beta1, W2, b2, gamma2, beta2, src, dst):
    features = np.asarray(features, np.float32)
    W1 = np.asarray(W1, np.float32); b1 = np.asarray(b1, np.float32)
    W2 = np.asarray(W2, np.float32); b2 = np.asarray(b2, np.float32)
    gamma1 = np.asarray(gamma1, np.float32); beta1 = np.asarray(beta1, np.float32)
    gamma2 = np.asarray(gamma2, np.float32); beta2 = np.asarray(beta2, np.float32)
    src = np.asarray(src, np.int32); dst = np.asarray(dst, np.int32)

    norm_src, norm_dst_sh, pad_idx, K = _prep(src, dst)
    try:
        return _device_impl(features, W1, b1, gamma1, beta1, W2, b2,
                            gamma2, beta2, norm_src, norm_dst_sh, pad_idx, K)
    except Exception as e:  # device path unavailable -> correct host fallback
        import sys
        print(f"kernel: device path failed ({e!r}); host fallback", file=sys.stderr)
        return _host_impl(features, W1, b1, gamma1, beta1, W2, b2, gamma2,
                          beta2, src, dst, norm_src, norm_dst_sh.reshape(N))



# revision 9
# speedup vs baseline: 5.1474x; 5.1474x over previous
"""GCN (2x GraphConv + BatchNorm) as a Bass/Tile kernel on 8 Trainium2 cores.

Sharding: 1D node partition (NS = N/8 dst-nodes per core).

Per layer:
  1. transform: z_local = (x_local * norm_src) @ W          (PE, per-core shard)
  2. AllGather z -> Z[N, H] (replicated message table in HBM)
  3. aggregation: per-core edges grouped by (dst-quarter, src-chunk);
     dma_gather Z[src] -> SBUF msgs, dma_scatter_add msgs -> agg[dst] (HBM).
     src-chunk grouping keeps gather indices < 32768 (int16 HW limit);
     dst-quarter grouping gives 4 independent scatter chains (disjoint HBM
     ranges -> no RMW races, Tile runs them concurrently on 4 SWDGE queues).
  4. finalize: agg * norm_dst + b, ELU  (DVE/ACT)
  5. BatchNorm: local partial sums -> AllReduce -> affine apply.

Host preprocessing (edge sort, index layout) is cached keyed on a content
hash of src/dst; the compiled jit + device-resident index tensors are reused
across calls.
"""

import hashlib
import os
import sys
from contextlib import ExitStack
from functools import partial

import numpy as np

N = 100000
E = 1600000
F = 128
H = 64
EPS = 1e-5
NC = 8
NS = N // NC                     # 12500 nodes per core
P = 128

NCHUNK = 4                       # src chunks for int16 gather indices
CHUNK = N // NCHUNK              # 25000 rows per gather window
NQ = 4                           # dst quarters (independent scatter chains)
QROWS = 3200                     # nodes per quarter (multiple of 128)
QSTRIDE = QROWS + 128            # +junk zone for padding scatters
JUNK = QROWS + 64                # junk row (relative to quarter view)
NS_PAD = ((NS + 127) // 128) * 128   # 12544
TILES = NS_PAD // 128            # 98
QTILES = QROWS // 128            # 25 tiles per quarter (last quarter fewer)
NG = NQ * NCHUNK                 # 16 edge groups per layer


def _agg_row(t):
    """DRAM row in agg for node-tile t (tiles grouped by quarter stripes)."""
    q, qt = divmod(t, QTILES)
    return q * QSTRIDE + qt * 128


def _wrap_idx(a):
    """int16 index array [n] -> SWDGE SBUF layout [128, n//16].

    Index i lives at partition i%16, column i//16; replicated 8x across the
    128 partitions (one copy per Q7 core).
    """
    n = a.shape[0]
    assert n % 16 == 0
    w = a.reshape(n // 16, 16).T.astype(np.int16)      # [16, n//16]
    return np.tile(w, (8, 1))                          # [128, n//16]


def _prep(src, dst):
    """Host-side graph preprocessing (cached per graph)."""
    deg_out = np.bincount(src, minlength=N).astype(np.float32)
    deg_in = np.bincount(dst, minlength=N).astype(np.float32)
    norm_src = 1.0 / np.sqrt(np.maximum(deg_out, 1.0))
    norm_dst = 1.0 / np.sqrt(np.maximum(deg_in, 1.0))

    core = dst // NS
    ld = dst - core * NS
    q = ld // QROWS
    c = src // CHUNK
    gkey = (core * NQ + q) * NCHUNK + c
    order = np.argsort(gkey * np.int64(N) + src, kind="stable")
    gkey_s = gkey[order]
    src_s = src[order]
    ld_s = ld[order]

    counts = np.bincount(gkey_s, minlength=NC * NG)
    S_G = int(((counts.max() + 127) // 128) * 128)

    # padded per-group index arrays
    gidx = np.zeros((NC, NG, S_G), np.int16)           # pad: gather Z row 0
    sidx = np.full((NC, NG, S_G), JUNK, np.int16)      # pad: scatter to junk
    starts = np.concatenate([[0], np.cumsum(counts)])
    gi_flat = (src_s % CHUNK).astype(np.int16)
    si_flat = (ld_s % QROWS).astype(np.int16)
    for k in range(NC * NG):
        s, e = starts[k], starts[k + 1]
        cc, g = divmod(k, NG)
        gidx[cc, g, : e - s] = gi_flat[s:e]
        sidx[cc, g, : e - s] = si_flat[s:e]

    gidx_w = np.zeros((NC, NG, 128, S_G // 16), np.int16)
    sidx_w = np.zeros((NC, NG, 128, S_G // 16), np.int16)
    for cc in range(NC):
        for g in range(NG):
            gidx_w[cc, g] = _wrap_idx(gidx[cc, g])
            sidx_w[cc, g] = _wrap_idx(sidx[cc, g])

    def col_layout(v):
        # per-core [NS] -> [NC, 128, TILES] (node n -> partition n%128, col n//128)
        out = np.zeros((NC, NS_PAD), np.float32)
        out[:, :NS] = v.reshape(NC, NS)
        return np.ascontiguousarray(out.reshape(NC, TILES, 128).transpose(0, 2, 1))

    nsrc_col = col_layout(norm_src)
    ndst_col = col_layout(norm_dst)
    mask = np.zeros((128, 1), np.float32)
    mask[: NS - (TILES - 1) * 128, 0] = 1.0            # valid rows of last tile

    return dict(S_G=S_G, gidx=gidx_w, sidx=sidx_w,
                nsrc=nsrc_col, ndst=ndst_col, mask=mask)


def _build_nc(S_G):
    from concourse import bass, bacc, mybir, tile

    f32 = mybir.dt.float32
    i16 = mybir.dt.int16
    AF = mybir.ActivationFunctionType
    OP = mybir.AluOpType

    nc = bacc.Bacc(None, target_bir_lowering=False, debug=False,
                   num_swdge_queues=1)

    feats = nc.declare_dram_parameter("feats", [NS, F], f32, False)
    nsrc = nc.declare_dram_parameter("nsrc", [P, TILES], f32, False)
    ndst = nc.declare_dram_parameter("ndst", [P, TILES], f32, False)
    maskp = nc.declare_dram_parameter("maskp", [P, 1], f32, False)
    gidx = nc.declare_dram_parameter("gidx", [NG, P, S_G // 16], i16, False)
    sidx = nc.declare_dram_parameter("sidx", [NG, P, S_G // 16], i16, False)
    W1 = nc.declare_dram_parameter("W1", [F, H], f32, False)
    W2 = nc.declare_dram_parameter("W2", [H, H], f32, False)
    bgb = nc.declare_dram_parameter("bgb", [1, 6 * H], f32, False)  # b1,g1,be1,b2,g2,be2
    out = nc.declare_dram_parameter("out", [NS, H], f32, True)

    ident = nc.inline_tensor(np.eye(P, dtype=np.float32), "ident")

    z1l = nc.dram_tensor("z1l", [NS, H], f32)
    z2l = nc.dram_tensor("z2l", [NS, H], f32)
    Z1 = nc.dram_tensor("Z1", [N, H], f32, addr_space="Shared")
    Z2 = nc.dram_tensor("Z2", [N, H], f32, addr_space="Shared")
    agg1 = nc.dram_tensor("agg1", [NQ * QSTRIDE, H], f32)
    agg2 = nc.dram_tensor("agg2", [NQ * QSTRIDE, H], f32)
    bn1i = nc.dram_tensor("bn1i", [1, 2 * H], f32)
    bn2i = nc.dram_tensor("bn2i", [1, 2 * H], f32)
    bn1o = nc.dram_tensor("bn1o", [1, 2 * H], f32, addr_space="Shared")
    bn2o = nc.dram_tensor("bn2o", [1, 2 * H], f32, addr_space="Shared")

    groups = [list(range(NC))]

    with tile.TileContext(nc) as tc, ExitStack() as ctx:
        const = ctx.enter_context(tc.tile_pool(name="const", bufs=1))
        xio = ctx.enter_context(tc.tile_pool(name="xio", bufs=3))
        xtp = ctx.enter_context(tc.tile_pool(name="xtp", bufs=3))
        zio = ctx.enter_context(tc.tile_pool(name="zio", bufs=3))
        idxp = ctx.enter_context(tc.tile_pool(name="idxp", bufs=3))
        msgp = ctx.enter_context(tc.tile_pool(name="msgp", bufs=2))
        aggio = ctx.enter_context(tc.tile_pool(name="aggio", bufs=3))
        tmp = ctx.enter_context(tc.tile_pool(name="tmp", bufs=6))
        small = ctx.enter_context(tc.tile_pool(name="small", bufs=8))
        hres = ctx.enter_context(tc.tile_pool(name="hres", bufs=1))
        statp = ctx.enter_context(tc.tile_pool(name="statp", bufs=2))
        bcp = ctx.enter_context(tc.tile_pool(name="bcp", bufs=6))
        pst = ctx.enter_context(tc.tile_pool(name="pst", bufs=2, space="PSUM"))
        psz = ctx.enter_context(tc.tile_pool(name="psz", bufs=2, space="PSUM"))
        psb = ctx.enter_context(tc.tile_pool(name="psb", bufs=2, space="PSUM"))

        # ---- constants ----
        identt = const.tile([P, P], f32)
        nc.sync.dma_start(identt[:], ident[:])
        W1t = const.tile([F, H], f32)
        nc.sync.dma_start(W1t[:], W1[:])
        W2t = const.tile([H, H], f32)
        nc.sync.dma_start(W2t[:], W2[:])
        nsrct = const.tile([P, TILES], f32)
        nc.sync.dma_start(nsrct[:], nsrc[:])
        ndstt = const.tile([P, TILES], f32)
        nc.sync.dma_start(ndstt[:], ndst[:])
        maskt = const.tile([P, 1], f32)
        nc.sync.dma_start(maskt[:], maskp[:])
        bgbt = const.tile([1, 6 * H], f32)
        nc.sync.dma_start(bgbt[:], bgb[:])
        onest = const.tile([1, P], f32)
        nc.vector.memset(onest[:], 1.0)
        onecol = const.tile([P, 1], f32)
        nc.vector.memset(onecol[:], 1.0)
        epst = const.tile([1, 1], f32)
        nc.vector.memset(epst[:], EPS)

        # ---- zero both agg buffers ----
        zcols = QSTRIDE * H // P  # 1664
        zerot = const.tile([P, zcols], f32)
        nc.vector.memset(zerot[:], 0.0)
        for agg in (agg1, agg2):
            for q in range(NQ):
                view = agg[q * QSTRIDE:(q + 1) * QSTRIDE, :].rearrange(
                    "(p n) f -> p (n f)", p=P)
                nc.sync.dma_start(view, zerot[:])

        h1 = hres.tile([P, TILES, H], f32, tag="h1")
        h2 = hres.tile([P, TILES, H], f32, tag="h2")

        def transform(src_getter, Wt, wk, z_dram):
            """z_dram[0:NS] = (x * norm_src) @ W ; x tile from src_getter(t)."""
            for t in range(TILES):
                rows = min(128, NS - t * 128)
                xs = src_getter(t, rows)               # scaled [P, wk] SBUF tile
                pt = pst.tile([P, P], f32, tag="pt")
                nc.tensor.transpose(pt[:wk, :], xs[:], identt[:])
                xT = xtp.tile([P, P], f32, tag="xT")
                nc.vector.tensor_copy(xT[:wk, :], pt[:wk, :])
                zp = psz.tile([P, H], f32, tag="zp")
                nc.tensor.matmul(zp[:], xT[:wk, :], Wt[:])
                zt = zio.tile([P, H], f32, tag="zt")
                nc.vector.tensor_copy(zt[:], zp[:])
                nc.sync.dma_start(z_dram[t * 128:t * 128 + rows, :], zt[:rows, :])

        def l1_src(t, rows):
            xt = xio.tile([P, F], f32, tag="xt")
            if rows < 128:
                nc.vector.memset(xt[:], 0.0)
            nc.sync.dma_start(xt[:rows, :], feats[t * 128:t * 128 + rows, :])
            xs = xio.tile([P, F], f32, tag="xs")
            nc.vector.tensor_scalar_mul(xs[:], xt[:], nsrct[:, t:t + 1])
            return xs

        def edges(Z, agg):
            for g in range(NG):
                q, c = divmod(g, NCHUNK)
                git = idxp.tile([P, S_G // 16], i16, tag="git")
                nc.sync.dma_start(git[:], gidx[g])
                sit = idxp.tile([P, S_G // 16], i16, tag="sit")
                nc.sync.dma_start(sit[:], sidx[g])
                mt = msgp.tile([P, S_G // P, H], f32, tag="mt")
                nc.gpsimd.dma_gather(
                    mt[:], Z[c * CHUNK:(c + 1) * CHUNK, :], git[:],
                    S_G, S_G, H, queue_num=0)
                nc.gpsimd.dma_scatter_add(
                    agg[q * QSTRIDE:(q + 1) * QSTRIDE, :], mt[:], sit[:],
                    S_G, S_G, H, queue_num=0)

        def finalize(agg, bofs, hdst, bni, bno):
            """agg -> hdst = elu(agg*norm_dst + b); returns BN (A,C) bcast tiles."""
            bb = psb.tile([P, H], f32, tag="psb")
            nc.tensor.matmul(bb[:], onest[:], bgbt[:, bofs * H:(bofs + 1) * H])
            bbs = bcp.tile([P, H], f32, tag="bbs")
            nc.vector.tensor_copy(bbs[:], bb[:])
            acc = statp.tile([P, 2 * H], f32, tag="acc")
            nc.vector.memset(acc[:], 0.0)
            for t in range(TILES):
                row = _agg_row(t)
                at = aggio.tile([P, H], f32, tag="at")
                nc.sync.dma_start(at[:], agg[row:row + 128, :])
                ft = tmp.tile([P, H], f32, tag="ft")
                nc.vector.tensor_scalar_mul(ft[:], at[:], ndstt[:, t:t + 1])
                nc.vector.tensor_tensor(ft[:], ft[:], bbs[:], OP.add)
                rt = tmp.tile([P, H], f32, tag="rt")
                nc.scalar.activation(rt[:], ft[:], AF.Relu)
                et = tmp.tile([P, H], f32, tag="et")
                nc.vector.tensor_scalar_min(et[:], ft[:], 0.0)
                e2 = tmp.tile([P, H], f32, tag="e2")
                nc.scalar.activation(e2[:], et[:], AF.Exp)
                hs = hdst[:, t, :]
                nc.vector.tensor_tensor(hs, rt[:], e2[:], OP.add)
                nc.vector.tensor_scalar_add(hs, hs, -1.0)
                if t == TILES - 1:
                    hm = tmp.tile([P, H], f32, tag="hm")
                    nc.vector.tensor_scalar_mul(hm[:], hs, maskt[:, 0:1])
                    stat_src = hm[:]
                else:
                    stat_src = hs
                nc.vector.tensor_tensor(acc[:, :H], acc[:, :H], stat_src, OP.add)
                sq = tmp.tile([P, H], f32, tag="sq")
                nc.scalar.square(sq[:], stat_src)
                nc.vector.tensor_tensor(acc[:, H:], acc[:, H:], sq[:], OP.add)
            pacc = psb.tile([1, 2 * H], f32, tag="psb")
            nc.tensor.matmul(pacc[:], onecol[:], acc[:])
            accr = small.tile([1, 2 * H], f32, tag="accr")
            nc.vector.tensor_copy(accr[:], pacc[:])
            nc.sync.dma_start(bni[:], accr[:])
            nc.gpsimd.collective_compute(
                "AllReduce", OP.add, replica_groups=groups,
                ins=[bni[:]], outs=[bno[:]])
            st = small.tile([1, 2 * H], f32, tag="st")
            nc.sync.dma_start(st[:], bno[:])
            mean = small.tile([1, H], f32, tag="mean")
            nc.vector.tensor_scalar_mul(mean[:], st[:, :H], 1.0 / N)
            var = small.tile([1, H], f32, tag="var")
            nc.vector.tensor_scalar_mul(var[:], st[:, H:], 1.0 / N)
            msq = small.tile([1, H], f32, tag="msq")
            nc.scalar.square(msq[:], mean[:])
            nc.vector.tensor_tensor(var[:], var[:], msq[:], OP.subtract)
            sd = small.tile([1, H], f32, tag="sd")
            nc.scalar.activation(sd[:], var[:], AF.Sqrt, bias=epst[:])
            rs = small.tile([1, H], f32, tag="rs")
            nc.vector.reciprocal(rs[:], sd[:])
            A = small.tile([1, H], f32, tag="A")
            nc.vector.tensor_tensor(A[:], bgbt[:, (bofs + 1) * H:(bofs + 2) * H], rs[:], OP.mult)
            mA = small.tile([1, H], f32, tag="mA")
            nc.vector.tensor_tensor(mA[:], mean[:], A[:], OP.mult)
            C = small.tile([1, H], f32, tag="C")
            nc.vector.tensor_tensor(C[:], bgbt[:, (bofs + 2) * H:(bofs + 3) * H],
                                    mA[:], OP.subtract)
            pA = psb.tile([P, H], f32, tag="psb")
            nc.tensor.matmul(pA[:], onest[:], A[:])
            Ab = bcp.tile([P, H], f32, tag="Ab")
            nc.vector.tensor_copy(Ab[:], pA[:])
            pC = psb.tile([P, H], f32, tag="psb")
            nc.tensor.matmul(pC[:], onest[:], C[:])
            Cb = bcp.tile([P, H], f32, tag="Cb")
            nc.vector.tensor_copy(Cb[:], pC[:])
            return Ab, Cb

        # ================= layer 1 =================
        transform(l1_src, W1t, F, z1l)
        nc.gpsimd.collective_compute(
            "AllGather", OP.bypass, replica_groups=groups,
            ins=[z1l[:]], outs=[Z1[:]])
        edges(Z1, agg1)
        A1, C1 = finalize(agg1, 0, h1, bn1i, bn1o)

        # ================= layer 2 =================
        def l2_src(t, rows):
            xs = xio.tile([P, H], f32, tag="xs2")
            nc.vector.tensor_tensor(xs[:], h1[:, t, :], A1[:], OP.mult)
            nc.vector.tensor_tensor(xs[:], xs[:], C1[:], OP.add)
            nc.vector.tensor_scalar_mul(xs[:], xs[:], nsrct[:, t:t + 1])
            return xs

        transform(l2_src, W2t, H, z2l)
        nc.gpsimd.collective_compute(
            "AllGather", OP.bypass, replica_groups=groups,
            ins=[z2l[:]], outs=[Z2[:]])
        edges(Z2, agg2)
        A2, C2 = finalize(agg2, 3, h2, bn2i, bn2o)

        # ---- output: BN-apply layer-2 ----
        for t in range(TILES):
            rows = min(128, NS - t * 128)
            ot = tmp.tile([P, H], f32, tag="ot")
            nc.vector.tensor_tensor(ot[:], h2[:, t, :], A2[:], OP.mult)
            nc.vector.tensor_tensor(ot[:], ot[:], C2[:], OP.add)
            nc.sync.dma_start(out[t * 128:t * 128 + rows, :], ot[:rows, :])

    nc.compile()
    return nc


class _Runner:
    """Mirrors bass2jax.run_bass_via_pjrt with a cached jit + device-resident
    static inputs."""

    def __init__(self, nc, static_per_core):
        import jax
        import jax.numpy as jnp
        from jax.sharding import Mesh, PartitionSpec, NamedSharding
        from concourse import bass2jax, mybir

        try:
            from jax.experimental.shard_map import shard_map
        except ImportError:
            from jax import shard_map

        bass2jax.install_neuronx_cc_hook()

        self.jax = jax
        partition_name = (nc.partition_id_tensor.name
                          if nc.partition_id_tensor else None)
        in_names, out_names, out_avals, zero_outs = [], [], [], []
        for alloc in nc.m.functions[0].allocations:
            if not isinstance(alloc, mybir.MemoryLocationSet):
                continue
            name = alloc.memorylocations[0].name
            if alloc.kind == "ExternalInput":
                if name != partition_name:
                    in_names.append(name)
            elif alloc.kind == "ExternalOutput":
                out_names.append(name)
                shape = tuple(alloc.tensor_shape)
                dtype = mybir.dt.np(alloc.dtype)
                out_avals.append(jax.core.ShapedArray(shape, dtype))
                zero_outs.append(np.zeros(shape, dtype))
        n_params = len(in_names)
        n_outs = len(out_avals)
        all_names = list(in_names) + out_names
        if partition_name is not None:
            all_names.append(partition_name)
        self.in_names = in_names
        self.out_names = out_names
        self.zero_outs = zero_outs

        from concourse.bass2jax import _bass_exec_p, partition_id_tensor

        def _body(*args):
            operands = list(args)
            if partition_name is not None:
                operands.append(partition_id_tensor())
            outs = _bass_exec_p.bind(
                *operands,
                out_avals=tuple(out_avals),
                in_names=tuple(all_names),
                out_names=tuple(out_names),
                lowering_input_output_aliases=(),
                sim_require_finite=True,
                sim_require_nnan=True,
                nc=nc,
            )
            return tuple(outs)

        devices = jax.devices()[:NC]
        assert len(devices) == NC
        mesh = Mesh(np.asarray(devices), ("core",))
        in_specs = (PartitionSpec("core"),) * (n_params + n_outs)
        out_specs = (PartitionSpec("core"),) * n_outs
        donate = tuple(range(n_params, n_params + n_outs))
        self.sharded = jax.jit(
            shard_map(_body, mesh=mesh, in_specs=in_specs,
                      out_specs=out_specs, check_rep=False),
            donate_argnums=donate, keep_unused=True)

        # device-resident static inputs (concat over cores on axis 0)
        sh = NamedSharding(mesh, PartitionSpec("core"))
        self.static_dev = {}
        for name, arrs in static_per_core.items():
            glob = np.concatenate(arrs, axis=0)
            self.static_dev[name] = jax.device_put(glob, sh)

    def __call__(self, dynamic_global):
        args = []
        for name in self.in_names:
            if name in self.static_dev:
                args.append(self.static_dev[name])
            else:
                args.append(dynamic_global[name])
        for z in self.zero_outs:
            args.append(np.zeros((NC * z.shape[0],) + z.shape[1:], z.dtype))
        outs = self.sharded(*args)
        return {name: np.asarray(outs[i]) for i, name in enumerate(self.out_names)}


_CACHE = {}


def _graph_key(src, dst):
    hsh = hashlib.sha1()
    hsh.update(src.shape[0].to_bytes(8, "little"))
    hsh.update(np.ascontiguousarray(src[::997]).tobytes())
    hsh.update(np.ascontiguousarray(dst[::997]).tobytes())
    hsh.update(int(src.sum(dtype=np.int64)).to_bytes(16, "little", signed=True))
    hsh.update(int(dst.sum(dtype=np.int64)).to_bytes(16, "little", signed=True))
    return hsh.hexdigest()


def _get_state(src, dst):
    key = _graph_key(src, dst)
    st = _CACHE.get(key)
    if st is None:
        prep = _prep(src, dst)
        nc = _build_nc(prep["S_G"])
        static = {
            "nsrc": [prep["nsrc"][c] for c in range(NC)],
            "ndst": [prep["ndst"][c] for c in range(NC)],
            "maskp": [prep["mask"] for _ in range(NC)],
            "gidx": [prep["gidx"][c] for c in range(NC)],
            "sidx": [prep["sidx"][c] for c in range(NC)],
        }
        st = _Runner(nc, static)
        _CACHE[key] = st
    return st


def _host_impl(features, W1, b1, gamma1, beta1, W2, b2, gamma2, beta2, src, dst):
    deg_out = np.bincount(src, minlength=N).astype(np.float32)
    deg_in = np.bincount(dst, minlength=N).astype(np.float32)
    norm_src = 1.0 / np.sqrt(np.maximum(deg_out, 1.0))
    norm_dst = 1.0 / np.sqrt(np.maximum(deg_in, 1.0))

    def conv(x, W, b):
        h = (x * norm_src[:, None]) @ W
        order = np.argsort(dst, kind="stable")
        d_sorted = dst[order]
        msgs = h[src[order]]
        agg = np.zeros((N, h.shape[1]), np.float32)
        starts = np.searchsorted(d_sorted, np.arange(N))
        np.add.reduceat(msgs, starts, axis=0, out=agg)
        agg[np.diff(np.concatenate([starts, [E]])) == 0] = 0
        v = agg * norm_dst[:, None] + b
        return np.where(v > 0, v, np.expm1(np.minimum(v, 0)))

    def bn(x, gamma, beta):
        mean = x.mean(0)
        var = np.square(x - mean).mean(0)
        return (x - mean) / np.sqrt(var + EPS) * gamma + beta

    h1 = bn(conv(features, W1, b1), gamma1, beta1)
    return bn(conv(h1, W2, b2), gamma2, beta2)


def kernel(features, W1, b1, gamma1, beta1, W2, b2, gamma2, beta2, src, dst):
    features = np.asarray(features, np.float32)
    W1 = np.asarray(W1, np.float32)
    W2 = np.asarray(W2, np.float32)
    b1 = np.asarray(b1, np.float32)
    b2 = np.asarray(b2, np.float32)
    gamma1 = np.asarray(gamma1, np.float32)
    gamma2 = np.asarray(gamma2, np.float32)
    beta1 = np.asarray(beta1, np.float32)
    beta2 = np.asarray(beta2, np.float32)
    src = np.asarray(src, np.int32)
    dst = np.asarray(dst, np.int32)

    try:
        st = _get_state(src, dst)
        bgb = np.stack([b1, gamma1, beta1, b2, gamma2, beta2]).reshape(1, 6 * H)
        dynamic = {
            "feats": features,                                   # [N, F]
            "W1": np.tile(W1, (NC, 1)),
            "W2": np.tile(W2, (NC, 1)),
            "bgb": np.tile(bgb, (NC, 1)),
        }
        outs = st(dynamic)
        return outs["out"]                                       # [N, H]
    except Exception as e:
        import traceback
        traceback.print_exc()
        print(f"kernel: device path failed ({e!r}); host fallback",
              file=sys.stderr)
        return _host_impl(features, W1, b1, gamma1, beta1, W2, b2,
                          gamma2, beta2, src, dst)
